# revision 8
# baseline (speedup 1.0000x reference)
"""Causal single-head attention (B=4, T=2048, C=H=1024) on 8 TRN2 NeuronCores.

Sharding: core = (batch b, query half qh).  Each core computes attention for
1024 queries of one batch against all 2048 keys of that batch.  The host
passes x ROLLED so the core's own query rows are always rows [0, 1024) of its
input.  In rolled coordinates the causal mask is:
  - keys [0, 1024)   (own half):  triangular mask f >= p  (core independent)
  - keys [1024, 2048) (other half): all-keep or all-drop depending on which
    half this core owns.  Implemented as a per-core bias input (0 or -30)
    added inside the exp activation: exp(s - 30) ~ 5e-13 ~ 0.
Softmax uses no max subtraction (logits are ~N(0, 0.33), |s| < ~2, so exp is
fp32-safe); the denominator is obtained by a matmul with a ones vector and
normalization is reciprocal+multiply.  Fully-masked score tiles are skipped.

Matmul operands are float32r (PE fast fp32 mode, 1 cyc/row at N>=256 vs 4
for fp32).  Producers write the operand tiles directly as f32r (the engines
round); weight DRAM tensors are declared f32r (bit pass-through on DMA).
"""

import math
import sys

sys.path.insert(0, "/opt/trn_rl_repo")

import numpy as np

B, T, C, H = 4, 2048, 1024, 1024
NCORES = 8
TQ = T // 2          # queries per core
P = 128              # partitions
CC = C // P          # contraction chunks for projections
HC = H // P          # contraction chunks for scores
NTK = T // P         # key chunks (16)
TQT = 256            # tq tile width in attention phase
NJ = TQ // TQT       # 4 tq tiles
NQC = TQT // P       # 2 query chunks of 128 per tq tile
XBLK = 256           # x rows per transpose/projection block
SCALE = 1.0 / math.sqrt(H)
TAIL_BIAS = -30.0

# matmul operand dtype: "float32" (exact, 4 cyc/row) or "float32r" (fast)
MM_DTYPE = "float32r"


def _active_tkcs(j):
    """Key chunks contributing to tq tile j (rolled coords).

    Own-half chunks (tkc < 8) above the diagonal are fully masked -> skipped.
    Tail chunks (tkc >= 8) always run; the exp bias keeps or kills them.
    """
    return [tkc for tkc in range(NTK) if tkc >= NTK // 2 or tkc < 2 * (j + 1)]


def _attn_body(tc, out_ap, xr, wq, wk, wv, tailbias):
    import concourse.mybir as mybir
    from concourse.masks import make_identity

    nc = tc.nc
    f32 = mybir.dt.float32
    mmdt = getattr(mybir.dt, MM_DTYPE)

    from contextlib import ExitStack

    with ExitStack() as ctx:
        consts = ctx.enter_context(tc.tile_pool(name="consts", bufs=1))
        identity = consts.tile([P, P], f32)
        make_identity(nc, identity)
        ones_f32 = consts.tile([P, 2], f32)
        nc.vector.memset(ones_f32, 1.0)
        ones = consts.tile([P, 2], mmdt)
        nc.vector.tensor_copy(ones, ones_f32)
        tail_sb = consts.tile([P, 1], f32)
        nc.sync.dma_start(out=tail_sb, in_=tailbias)
        warm = consts.tile([P, 1], f32)
        nc.scalar.activation(warm, tail_sb, mybir.ActivationFunctionType.Exp)

        big = ctx.enter_context(tc.tile_pool(name="big", bufs=1))
        KT = big.tile([P, HC, T], mmdt)    # K^T: [h, tk], 64KB/part
        V = big.tile([P, NTK, H], mmdt)    # V: [tk, h], 64KB/part

        dram = ctx.enter_context(
            tc.tile_pool(name="dram", bufs=1, space="DRAM")
        )
        QTd = dram.tile([HC, P, TQ], mmdt)  # Q^T staging: [hc, h, tq]

        # ---------------- projection passes ----------------
        # One pass per weight; each pass re-streams x rows and PE-transposes
        # them into x^T blocks [c, t] which feed the matmuls.
        def sweep(w_dram, nblocks, emit_block):
            with ExitStack() as pctx:
                wpool = pctx.enter_context(tc.tile_pool(name="wpool", bufs=1))
                xrow_pool = pctx.enter_context(
                    tc.tile_pool(name="xrow", bufs=2)
                )
                xt_pool = pctx.enter_context(tc.tile_pool(name="xt", bufs=1))
                tpsum = pctx.enter_context(
                    tc.tile_pool(name="tpsum", bufs=4, space="PSUM")
                )
                ppsum = pctx.enter_context(
                    tc.tile_pool(name="ppsum", bufs=4, space="PSUM")
                )
                stage = pctx.enter_context(tc.tile_pool(name="pstage", bufs=2))

                w_sb = wpool.tile([P, CC, H], mmdt, tag="w")
                nc.sync.dma_start(
                    out=w_sb, in_=w_dram.rearrange("(cc p) h -> p cc h", p=P)
                )
                for tt in range(nblocks):
                    xt = xt_pool.tile([P, CC, XBLK], mmdt, tag="xt")
                    for r in range(XBLK // P):
                        xrow = xrow_pool.tile([P, C], f32, tag="xr")
                        row0 = tt * XBLK + r * P
                        nc.sync.dma_start(
                            out=xrow, in_=xr[row0 : row0 + P, :]
                        )
                        for cc in range(CC):
                            pt = tpsum.tile([P, P], f32, tag="tp")
                            nc.tensor.transpose(
                                pt, xrow[:, cc * P : (cc + 1) * P], identity
                            )
                            nc.any.tensor_copy(
                                xt[:, cc, r * P : (r + 1) * P], pt
                            )
                    emit_block(tt, w_sb, xt, ppsum, stage)

        # Q^T pass: blocks 0..3 (query rows), out [h, tq] -> DRAM
        def qt_block(tt, w_sb, xt, ppsum, stage):
            for hc in range(HC):
                ps = ppsum.tile([P, XBLK], f32, tag="pp")
                for cc in range(CC):
                    nc.tensor.matmul(
                        ps,
                        w_sb[:, cc, hc * P : (hc + 1) * P],
                        xt[:, cc, :],
                        start=(cc == 0),
                        stop=(cc == CC - 1),
                    )
                st = stage.tile([P, XBLK], mmdt, tag="st")
                nc.any.tensor_copy(st, ps)
                nc.sync.dma_start(
                    out=QTd[hc, :, tt * XBLK : (tt + 1) * XBLK], in_=st
                )

        # K^T pass: all 8 blocks, out [h, tk] resident
        def kt_block(tt, w_sb, xt, ppsum, stage):
            for hc in range(HC):
                ps = ppsum.tile([P, XBLK], f32, tag="pp")
                for cc in range(CC):
                    nc.tensor.matmul(
                        ps,
                        w_sb[:, cc, hc * P : (hc + 1) * P],
                        xt[:, cc, :],
                        start=(cc == 0),
                        stop=(cc == CC - 1),
                    )
                nc.any.tensor_copy(
                    KT[:, hc, tt * XBLK : (tt + 1) * XBLK], ps
                )

        # V pass: all 8 blocks, out [tk, h] resident
        def v_block(tt, w_sb, xt, ppsum, stage):
            for sub in range(XBLK // P):
                tkc = tt * (XBLK // P) + sub
                for ht in range(H // 512):
                    ps = ppsum.tile([P, 512], f32, tag="pv")
                    for cc in range(CC):
                        nc.tensor.matmul(
                            ps,
                            xt[:, cc, sub * P : (sub + 1) * P],
                            w_sb[:, cc, ht * 512 : (ht + 1) * 512],
                            start=(cc == 0),
                            stop=(cc == CC - 1),
                        )
                    nc.any.tensor_copy(
                        V[:, tkc, ht * 512 : (ht + 1) * 512], ps
                    )

        sweep(wq, TQ // XBLK, qt_block)
        sweep(wk, T // XBLK, kt_block)
        sweep(wv, T // XBLK, v_block)

        # ---------------- attention phase ----------------
        with ExitStack() as actx:
            qt_pool = actx.enter_context(tc.tile_pool(name="qt", bufs=2))
            et_pool = actx.enter_context(tc.tile_pool(name="et", bufs=2))
            spsum = actx.enter_context(
                tc.tile_pool(name="spsum", bufs=2, space="PSUM")
            )
            opsum = actx.enter_context(
                tc.tile_pool(name="opsum", bufs=4, space="PSUM")
            )
            dpsum = actx.enter_context(
                tc.tile_pool(name="dpsum", bufs=2, space="PSUM")
            )
            small = actx.enter_context(tc.tile_pool(name="small", bufs=4))
            ostage = actx.enter_context(tc.tile_pool(name="ostage", bufs=4))

            for j in range(NJ):
                q0 = j * TQT
                qt = qt_pool.tile([P, HC, TQT], mmdt, tag="qt")
                nc.sync.dma_start(
                    out=qt,
                    in_=QTd[:, :, q0 : q0 + TQT].rearrange("hc p t -> p hc t"),
                )
                et = et_pool.tile([P, NTK, TQT], mmdt, tag="et")
                actives = _active_tkcs(j)
                o_ps = [
                    [
                        opsum.tile([P, 512], f32, tag="op", name=f"o_{qc}_{ht}")
                        for ht in range(2)
                    ]
                    for qc in range(NQC)
                ]
                d_ps = [
                    dpsum.tile([P, 2], f32, tag="dp", name=f"d_{qc}")
                    for qc in range(NQC)
                ]

                def consume(i, tkc):
                    # O and denominator accumulation for key chunk tkc
                    last = i == len(actives) - 1
                    for qc in range(NQC):
                        lhs = et[:, tkc, qc * P : (qc + 1) * P]
                        for ht in range(2):
                            nc.tensor.matmul(
                                o_ps[qc][ht],
                                lhs,
                                V[:, tkc, ht * 512 : (ht + 1) * 512],
                                start=(i == 0),
                                stop=last,
                            )
                        nc.tensor.matmul(
                            d_ps[qc],
                            lhs,
                            ones,
                            start=(i == 0),
                            stop=last,
                        )

                for i, tkc in enumerate(actives):
                    # scores S^T[tk, tq] for this key chunk
                    sp = spsum.tile([P, TQT], f32, tag="sp")
                    for hc in range(HC):
                        nc.tensor.matmul(
                            sp,
                            KT[:, hc, tkc * P : (tkc + 1) * P],
                            qt[:, hc, :],
                            start=(hc == 0),
                            stop=(hc == HC - 1),
                        )
                    # exp with fused scale; tail chunks get the per-core bias
                    bias = tail_sb if tkc >= NTK // 2 else 0.0
                    nc.scalar.activation(
                        et[:, tkc, :],
                        sp,
                        mybir.ActivationFunctionType.Exp,
                        bias=bias,
                        scale=SCALE,
                    )
                    # diagonal-crossing tiles: triangular mask in rolled coords
                    if tkc in (2 * j, 2 * j + 1):
                        nc.gpsimd.affine_select(
                            out=et[:, tkc, :],
                            in_=et[:, tkc, :],
                            compare_op=mybir.AluOpType.is_ge,
                            fill=0.0,
                            base=TQT * j - P * tkc,
                            pattern=[[1, TQT]],
                            channel_multiplier=-1,
                        )
                    consume(i, tkc)

                # normalize + write out
                for qc in range(NQC):
                    rec = small.tile([P, 1], f32, tag="rec")
                    nc.vector.reciprocal(rec, d_ps[qc][:, 0:1])
                    for ht in range(2):
                        ot = ostage.tile([P, 512], f32, tag="ot")
                        nc.vector.tensor_scalar_mul(ot, o_ps[qc][ht], rec)
                        nc.sync.dma_start(
                            out=out_ap[
                                q0 + qc * P : q0 + (qc + 1) * P,
                                ht * 512 : (ht + 1) * 512,
                            ],
                            in_=ot,
                        )


def build_nc():
    import concourse.mybir as mybir
    import concourse.tile as tile
    from concourse import bacc

    nc = bacc.Bacc(
        "TRN2",
        target_bir_lowering=False,
        debug=False,
        num_devices=NCORES,
    )
    f32 = mybir.dt.float32
    mmdt = getattr(mybir.dt, MM_DTYPE)
    xr = nc.dram_tensor("xr", [T, C], f32, kind="ExternalInput").ap()
    wq = nc.dram_tensor("wq", [C, H], mmdt, kind="ExternalInput").ap()
    wk = nc.dram_tensor("wk", [C, H], mmdt, kind="ExternalInput").ap()
    wv = nc.dram_tensor("wv", [C, H], mmdt, kind="ExternalInput").ap()
    tailbias = nc.dram_tensor(
        "tailbias", [P, 1], f32, kind="ExternalInput"
    ).ap()
    out = nc.dram_tensor("out", [TQ, H], f32, kind="ExternalOutput").ap()
    with tile.TileContext(nc) as tc:
        _attn_body(tc, out, xr, wq, wk, wv, tailbias)
    nc.compile()
    return nc


def make_in_maps(x, Wk, Wq, Wv):
    x = np.asarray(x, dtype=np.float32)
    Wk = np.ascontiguousarray(np.asarray(Wk, dtype=np.float32))
    Wq = np.ascontiguousarray(np.asarray(Wq, dtype=np.float32))
    Wv = np.ascontiguousarray(np.asarray(Wv, dtype=np.float32))
    in_maps = []
    for c in range(NCORES):
        b, qh = divmod(c, 2)
        xb = x[b]
        if qh == 0:
            xr = np.ascontiguousarray(xb)
            tail = np.full((P, 1), TAIL_BIAS, dtype=np.float32)
        else:
            xr = np.ascontiguousarray(
                np.concatenate([xb[TQ:], xb[:TQ]], axis=0)
            )
            tail = np.zeros((P, 1), dtype=np.float32)
        in_maps.append(
            {"xr": xr, "wq": Wq, "wk": Wk, "wv": Wv, "tailbias": tail}
        )
    return in_maps


def assemble_out(results):
    out = np.empty((B, T, H), dtype=np.float32)
    for c in range(NCORES):
        b, qh = divmod(c, 2)
        out[b, qh * TQ : (qh + 1) * TQ] = results[c]["out"]
    return out


def kernel(x, Wk, Wq, Wv):
    from concourse import bass_utils

    nc = build_nc()
    in_maps = make_in_maps(x, Wk, Wq, Wv)
    res = bass_utils.run_bass_kernel_spmd(
        nc, in_maps, core_ids=list(range(NCORES))
    )
    return assemble_out(res.results)
